# revision 22
# baseline (speedup 1.0000x reference)
"""Trainium2 Bass kernel for nn_Branch1_block (gnn_message_passing).

Data-parallel over batch on 8 NeuronCores (4 batches/core).

Pipeline per core (single compiled graph):
  host:    SE attention scalars, cheb2, blockdiag weights precomputed in numpy
  gconv1:  z_k = x @ cheb_k in bf16 (k=1,2; k=0 from DRAM row-major x),
           att folded as per-partition scale, feature mix in f32r producing
           transposed xg1T [n x (bt,o)] in bf16.
  gconv2:  batch-outer / n-chunk-inner so each batch's output is complete
           early; graph matmuls bf16 (lhsT = xg1T), k=0 via PE transposes,
           feature mix f32r; xg2 stays in SBUF (no DRAM spill).
  tail:    per batch, interleaved under the next batch's gconv2 compute:
           temporal convs as block matmuls (tconv1 bf16, tconv2/residual
           f32r), LayerNorm over nodes fused via ACT Identity(scale,bias).
"""
import sys

import numpy as np

try:
    import concourse.bass as bass
except ImportError:  # pragma: no cover - grading env fallback
    for p in ("/root/.axon_site", "/root/.axon_site/_ro/trn_rl_repo",
              "/root/.axon_site/_ro/pypackages", "/opt/trn_rl_repo"):
        if p not in sys.path:
            sys.path.append(p)
    import concourse.bass as bass

from contextlib import ExitStack

import ml_dtypes
import concourse.mybir as mybir
import concourse.tile as tile
from concourse import bacc
from concourse.bass_utils import run_bass_kernel_spmd

B, T, F, O, N, K = 32, 12, 16, 32, 2048, 3
NCORES = 8
BC = B // NCORES          # 4 batches per core
BT = BC * T               # 48
R1 = BT * F               # 768 rows (bt,f)
R2 = BT * O               # 1536 rows (bt,o)
J1 = R1 // 128            # 6
J2 = R2 // 128            # 12
NT = N // 128             # 16
NCH1 = 256                # gconv1 n-chunk
NNC1 = N // NCH1          # 8
NCH2 = 256                # gconv2 n-chunk
NNC2 = N // NCH2          # 8
TCH = 256                 # tail n-chunk
NTC = N // TCH            # 8

f32 = mybir.dt.float32
f32r = mybir.dt.float32r
bf16 = mybir.dt.bfloat16
AF = mybir.ActivationFunctionType
AX = mybir.AxisListType
ALU = mybir.AluOpType

# (jt, ji) pairs with nonzero temporal-conv block matrices
TC_PAIRS = {0: (0, 2), 1: (0, 1), 2: (1, 2)}

_compiled = {}


def _build(zero_lnb):
    nc = bacc.Bacc("TRN2", target_bir_lowering=False, debug=False)

    xT_d = nc.dram_tensor("xT", [N, R1], bf16, kind="ExternalInput").ap()
    xb_d = nc.dram_tensor("xb", [R1, N], f32, kind="ExternalInput").ap()
    c1b_d = nc.dram_tensor("c1b", [N, N], bf16, kind="ExternalInput").ap()
    c2b_d = nc.dram_tensor("c2b", [N, N], bf16, kind="ExternalInput").ap()
    th1_d = nc.dram_tensor("th1", [3, 128, 256], f32, kind="ExternalInput").ap()
    th2_d = nc.dram_tensor("th2", [3, 128, 128], f32, kind="ExternalInput").ap()
    th2b_d = nc.dram_tensor("th2b", [128, 128], bf16, kind="ExternalInput").ap()
    tcb1_d = nc.dram_tensor("tcb1", [3, 3, 128, 128], bf16,
                            kind="ExternalInput").ap()
    tcb2_d = nc.dram_tensor("tcb2", [3, 3, 128, 128], bf16,
                            kind="ExternalInput").ap()
    resbd_d = nc.dram_tensor("resbd", [2, 128, 128], f32, kind="ExternalInput").ap()
    identb_d = nc.dram_tensor("identb", [128, 128], bf16, kind="ExternalInput").ap()
    attc_d = nc.dram_tensor("attc", [128, 8], f32, kind="ExternalInput").ap()
    lng_d = nc.dram_tensor("lng", [128, N], f32, kind="ExternalInput").ap()
    lnb_d = nc.dram_tensor("lnb", [128, N], f32, kind="ExternalInput").ap()
    bias3_d = nc.dram_tensor("bias3", [128, 4], f32, kind="ExternalInput").ap()
    y_d = nc.dram_tensor("y", [R2, N], f32, kind="ExternalOutput").ap()

    dma = nc.sync.dma_start        # SP HWDGE queue
    dmaa = nc.scalar.dma_start     # Activation HWDGE queue (parallel to SP)
    dmag = nc.gpsimd.dma_start     # Pool SWDGE queue

    with tile.TileContext(nc) as tc, ExitStack() as top:
        cpool = top.enter_context(tc.tile_pool(name="const", bufs=1))
        identb_sb = cpool.tile([128, 128], bf16)
        th2_sb = cpool.tile([128, 3 * 128], f32r)
        th2b_sb = cpool.tile([128, 128], bf16)
        attc_sb = cpool.tile([128, 8], f32)
        zeros_sb = cpool.tile([128, TCH], f32)
        dma(identb_sb[:], identb_d)
        dma(th2_sb[:].rearrange("p (k c) -> p k c", c=128),
            th2_d.rearrange("k p c -> p k c").bitcast(f32r))
        dma(th2b_sb[:], th2b_d)
        dmaa(attc_sb[:], attc_d)
        nc.vector.memset(zeros_sb[:], 0.0)
        # tail constants, loaded up-front on the GPSIMD queue so they never
        # contend with the compute-gating loads
        tcb1_sb = cpool.tile([128, 9 * 128], bf16)
        tcb2_sb = cpool.tile([128, 9 * 128], bf16)
        resbd_sb = cpool.tile([128, 2 * 128], f32r)
        lng_sb = cpool.tile([128, N], f32)
        bias3_sb = cpool.tile([128, 4], f32)
        dmag(tcb1_sb[:].rearrange("p (i q) -> p i q", q=128),
             tcb1_d.rearrange("a b p q -> p (a b) q"))
        dmag(tcb2_sb[:].rearrange("p (i q) -> p i q", q=128),
             tcb2_d.rearrange("a b p q -> p (a b) q"))
        dmag(resbd_sb[:].rearrange("p (h q) -> p h q", q=128),
             resbd_d.rearrange("h p q -> p h q").bitcast(f32r))
        dmag(lng_sb[:], lng_d)
        dmag(bias3_sb[:], bias3_d)
        if not zero_lnb:
            lnb_sb = cpool.tile([128, N], f32)
            dmag(lnb_sb[:], lnb_d)

        xg1Tp = top.enter_context(tc.tile_pool(name="xg1T", bufs=1))
        xg1T_sb = xg1Tp.tile([128, NT, R2], bf16)

        # ---------- gconv1 ----------
        with tc.tile_pool(name="xTp", bufs=1) as xTpool, \
             tc.tile_pool(name="c1", bufs=1) as c1pool, \
             tc.tile_pool(name="chp", bufs=2) as chpool, \
             tc.tile_pool(name="g1sb", bufs=2) as g1pool, \
             tc.tile_pool(name="zps", bufs=3, space="PSUM") as zpsum, \
             tc.tile_pool(name="fps", bufs=2, space="PSUM") as fpsum:
            th1_sb = c1pool.tile([128, 3 * 256], f32r)
            dma(th1_sb[:].rearrange("p (k c) -> p k c", c=256),
                th1_d.rearrange("k p c -> p k c").bitcast(f32r))
            xT_sb = xTpool.tile([128, NT, R1], bf16)
            xTv = xT_d.rearrange("(mi p) r -> mi p r", p=128)
            for q in range(8):
                dm = dma if q % 2 == 0 else dmaa
                dm(xT_sb[:, q * 2:(q + 1) * 2, :],
                   xTv[q * 2:(q + 1) * 2].rearrange("mi p r -> p mi r"))

            for nci in range(NNC1):
                ncs = nci * NCH1
                ch1 = chpool.tile([128, NT, NCH1], bf16, tag="ch1", bufs=2,
                                  name=f"ch1_{nci}")
                ch2 = chpool.tile([128, NT, NCH1], bf16, tag="ch2", bufs=2,
                                  name=f"ch2_{nci}")
                for cd, ct, dm in ((c1b_d, ch1, dma), (c2b_d, ch2, dmaa)):
                    chv = cd[:, ncs:ncs + NCH1].rearrange("(mi p) n -> mi p n",
                                                          p=128)
                    for q in range(2):
                        dm(ct[:, q * 8:(q + 1) * 8, :],
                           chv[q * 8:(q + 1) * 8].rearrange("mi p n -> p mi n"))
                zk = {}
                for k, ch in ((1, ch1), (2, ch2)):
                    for j1 in range(J1):
                        zps = zpsum.tile([128, NCH1], f32)
                        for mi in range(NT):
                            nc.tensor.matmul(zps[:],
                                             xT_sb[:, mi, j1 * 128:(j1 + 1) * 128],
                                             ch[:, mi, :],
                                             start=(mi == 0), stop=(mi == NT - 1))
                        zt = g1pool.tile([128, NCH1], f32r, tag=f"z{k}",
                                         bufs=(7 if k == 1 else 3))
                        nc.vector.tensor_scalar_mul(zt[:], zps[:],
                                                    attc_sb[:, j1:j1 + 1])
                        zk[(k, j1)] = zt
                        if k == 2:
                            z0t = g1pool.tile([128, NCH1], f32r, tag="z0", bufs=3)
                            dma(z0t[:], xb_d[j1 * 128:(j1 + 1) * 128,
                                             ncs:ncs + NCH1].bitcast(f32r))
                            nc.vector.tensor_scalar_mul(z0t[:], z0t[:].bitcast(f32),
                                                        attc_sb[:, j1:j1 + 1])
                            for ntl in range(NCH1 // 128):
                                ntile = (ncs // 128) + ntl
                                fps = fpsum.tile([128, 256], f32)
                                nc.tensor.matmul(fps[:], z0t[:, ntl * 128:(ntl + 1) * 128],
                                                 th1_sb[:, 0:256], start=True, stop=False)
                                nc.tensor.matmul(fps[:],
                                                 zk[(1, j1)][:, ntl * 128:(ntl + 1) * 128],
                                                 th1_sb[:, 256:512],
                                                 start=False, stop=False)
                                nc.tensor.matmul(fps[:],
                                                 zk[(2, j1)][:, ntl * 128:(ntl + 1) * 128],
                                                 th1_sb[:, 512:768],
                                                 start=False, stop=True)
                                nc.scalar.activation(
                                    xg1T_sb[:, ntile, j1 * 256:(j1 + 1) * 256],
                                    fps[:], AF.Relu)

        # ---------- gconv2 + tail, interleaved per batch ----------
        with tc.tile_pool(name="chp2", bufs=2) as chpool2, \
             tc.tile_pool(name="g2sb", bufs=2) as g2pool, \
             tc.tile_pool(name="xg2p", bufs=2) as xg2pool, \
             tc.tile_pool(name="tlsb", bufs=3) as tlpool, \
             tc.tile_pool(name="tbig", bufs=2) as tbpool, \
             tc.tile_pool(name="tstat", bufs=2) as stpool:
            for b in range(BC):
                xg2b = [xg2pool.tile([128, N], bf16, tag="xg2", bufs=5,
                                     name=f"xg2_{b}_{i}") for i in range(3)]
                g2ps = ExitStack()
                zpsum2 = g2ps.enter_context(
                    tc.tile_pool(name=f"zps2_{b}", bufs=2, space="PSUM"))
                tpsumT = g2ps.enter_context(
                    tc.tile_pool(name=f"tpsT_{b}", bufs=2, space="PSUM"))
                fpsum2 = g2ps.enter_context(
                    tc.tile_pool(name=f"fps2_{b}", bufs=2, space="PSUM"))
                for nci in range(NNC2):
                    ncs = nci * NCH2
                    cb1 = chpool2.tile([128, NT, NCH2], bf16, tag="cb1", bufs=2,
                                       name=f"cb1_{b}_{nci}")
                    cb2 = chpool2.tile([128, NT, NCH2], bf16, tag="cb2", bufs=2,
                                       name=f"cb2_{b}_{nci}")
                    for cd, ct, dm in ((c1b_d, cb1, dma), (c2b_d, cb2, dmaa)):
                        chv = cd[:, ncs:ncs + NCH2].rearrange(
                            "(mi p) n -> mi p n", p=128)
                        for q in range(2):
                            dm(ct[:, q * 8:(q + 1) * 8, :],
                               chv[q * 8:(q + 1) * 8].rearrange("mi p n -> p mi n"))
                    zrec = {}
                    xg1r = {}
                    for jl in range(3):
                        j2 = 3 * b + jl
                        for k, cb in ((1, cb1), (2, cb2)):
                            zps = zpsum2.tile([128, NCH2], f32)
                            for mi in range(NT):
                                nc.tensor.matmul(
                                    zps[:], xg1T_sb[:, mi, j2 * 128:(j2 + 1) * 128],
                                    cb[:, mi, :],
                                    start=(mi == 0), stop=(mi == NT - 1))
                            zt = g2pool.tile([128, NCH2], f32r, tag=f"z2_{k}",
                                             bufs=4)
                            nc.vector.tensor_copy(zt[:], zps[:])
                            zrec[(jl, k)] = zt
                        xr = g2pool.tile([128, NCH2], bf16, tag="xg1r", bufs=4)
                        for tl in range(NCH2 // 128):
                            mi = (ncs // 128) + tl
                            tp = tpsumT.tile([128, 128], bf16)
                            nc.tensor.transpose(
                                tp[:], xg1T_sb[:, mi, j2 * 128:(j2 + 1) * 128],
                                identb_sb[:])
                            nc.vector.tensor_copy(xr[:, tl * 128:(tl + 1) * 128],
                                                  tp[:])
                        xg1r[jl] = xr
                    for jl in range(3):
                        fps = fpsum2.tile([128, NCH2], f32)
                        nc.tensor.matmul(fps[:], th2b_sb[:], xg1r[jl][:],
                                         start=True, stop=False)
                        nc.tensor.matmul(fps[:], th2_sb[:, 128:256],
                                         zrec[(jl, 1)][:], start=False, stop=False)
                        nc.tensor.matmul(fps[:], th2_sb[:, 256:384],
                                         zrec[(jl, 2)][:], start=False, stop=True)
                        nc.scalar.activation(xg2b[jl][:, ncs:ncs + NCH2],
                                             fps[:], AF.Relu)

                # ---- tail for batch b ----
                g2ps.close()
                tlps = ExitStack()
                tpsum1 = tlps.enter_context(
                    tc.tile_pool(name=f"tps1_{b}", bufs=2, space="PSUM"))
                tpsum2 = tlps.enter_context(
                    tc.tile_pool(name=f"tps2_{b}", bufs=2, space="PSUM"))
                rpsum = tlps.enter_context(
                    tc.tile_pool(name=f"rps_{b}", bufs=2, space="PSUM"))
                xt1b = [tbpool.tile([128, N], bf16, tag="xt1", bufs=3,
                                    name=f"xt1_{b}_{i}") for i in range(3)]
                xresl = []
                for jt in range(3):
                    jx = (3 * b + jt) // 2
                    xres = tbpool.tile([128, N], f32r, tag="xres", bufs=3,
                                       name=f"xres_{b}_{jt}")
                    dma(xres[:], xb_d[jx * 128:(jx + 1) * 128, :].bitcast(f32r))
                    xresl.append(xres)
                for jt in range(3):
                    ja, jb = TC_PAIRS[jt]
                    for ncc in range(NTC):
                        ncs = ncc * TCH
                        tp1 = tpsum1.tile([128, TCH], f32)
                        nc.tensor.matmul(tp1[:],
                                         tcb1_sb[:, (jt * 3 + ja) * 128:
                                                 (jt * 3 + ja + 1) * 128],
                                         xg2b[ja][:, ncs:ncs + TCH],
                                         start=True, stop=False)
                        nc.tensor.matmul(tp1[:],
                                         tcb1_sb[:, (jt * 3 + jb) * 128:
                                                 (jt * 3 + jb + 1) * 128],
                                         xg2b[jb][:, ncs:ncs + TCH],
                                         start=False, stop=True)
                        if ncc % 2 == 0:
                            nc.scalar.activation(xt1b[jt][:, ncs:ncs + TCH],
                                                 tp1[:], AF.Relu,
                                                 bias=bias3_sb[:, 0:1])
                        else:
                            nc.vector.scalar_tensor_tensor(
                                xt1b[jt][:, ncs:ncs + TCH], tp1[:],
                                bias3_sb[:, 0:1], zeros_sb[:],
                                ALU.add, ALU.max)
                for jt in range(3):
                    j2 = 3 * b + jt
                    ja, jb = TC_PAIRS[jt]
                    half = j2 % 2
                    xres = xresl[jt]
                    yfull = tbpool.tile([128, N], f32, tag="yfull", bufs=2,
                                        name=f"yf_{b}_{jt}")
                    for ncc in range(NTC):
                        ncs = ncc * TCH
                        tp2 = tpsum2.tile([128, TCH], f32)
                        nc.tensor.matmul(tp2[:],
                                         tcb2_sb[:, (jt * 3 + ja) * 128:
                                                 (jt * 3 + ja + 1) * 128],
                                         xt1b[ja][:, ncs:ncs + TCH],
                                         start=True, stop=False)
                        nc.tensor.matmul(tp2[:],
                                         tcb2_sb[:, (jt * 3 + jb) * 128:
                                                 (jt * 3 + jb + 1) * 128],
                                         xt1b[jb][:, ncs:ncs + TCH],
                                         start=False, stop=True)
                        xt2c = tlpool.tile([128, TCH], f32, tag="xt2c", bufs=3)
                        if ncc % 2 == 0:
                            nc.scalar.activation(xt2c[:], tp2[:], AF.Relu,
                                                 bias=bias3_sb[:, 1:2])
                        else:
                            nc.vector.scalar_tensor_tensor(
                                xt2c[:], tp2[:], bias3_sb[:, 1:2], zeros_sb[:],
                                ALU.add, ALU.max)
                        rp = rpsum.tile([128, TCH], f32)
                        nc.tensor.matmul(rp[:],
                                         resbd_sb[:, half * 128:(half + 1) * 128],
                                         xres[:, ncs:ncs + TCH],
                                         start=True, stop=True)
                        nc.vector.scalar_tensor_tensor(yfull[:, ncs:ncs + TCH],
                                                       rp[:], bias3_sb[:, 2:3],
                                                       xt2c[:], ALU.add, ALU.add)
                    # LayerNorm over n (free axis) + relu
                    ssum = stpool.tile([128, 1], f32, tag="ssum")
                    nc.vector.reduce_sum(ssum[:], yfull[:], axis=AX.X)
                    scr = tbpool.tile([128, N], f32, tag="scr", bufs=2,
                                      name=f"scr_{b}_{jt}")
                    sqs = stpool.tile([128, 1], f32, tag="sqs")
                    nc.scalar.activation(scr[:], yfull[:], AF.Square,
                                         accum_out=sqs[:])
                    mu = stpool.tile([128, 1], f32, tag="mu")
                    nc.vector.tensor_scalar_mul(mu[:], ssum[:], 1.0 / N)
                    musq = stpool.tile([128, 1], f32, tag="musq")
                    nc.vector.tensor_mul(musq[:], mu[:], mu[:])
                    var = stpool.tile([128, 1], f32, tag="var")
                    nc.vector.tensor_scalar(var[:], sqs[:], 1.0 / N, None, ALU.mult)
                    nc.vector.tensor_sub(var[:], var[:], musq[:])
                    nc.vector.tensor_scalar_add(var[:], var[:], 1e-5)
                    sd = stpool.tile([128, 1], f32, tag="sd")
                    nc.scalar.sqrt(sd[:], var[:])
                    istd = stpool.tile([128, 1], f32, tag="istd")
                    nc.vector.reciprocal(istd[:], sd[:])
                    nmi = stpool.tile([128, 1], f32, tag="nmi")
                    nc.vector.scalar_tensor_tensor(nmi[:], mu[:], -1.0, istd[:],
                                                   ALU.mult, ALU.mult)
                    # t = yfull*istd - mu*istd (DVE), *g on GPSIMD, relu on ACT
                    nc.vector.tensor_scalar(scr[:], yfull[:], istd[:], nmi[:],
                                            ALU.mult, ALU.add)
                    nc.gpsimd.tensor_mul(scr[:], scr[:], lng_sb[:])
                    if not zero_lnb:
                        nc.gpsimd.tensor_add(scr[:], scr[:], lnb_sb[:])
                    nc.scalar.activation(scr[:], scr[:], AF.Relu)
                    dmag(y_d[j2 * 128:(j2 + 1) * 128, :], scr[:])
                tlps.close()

    nc.compile()
    return nc


def _host_prep(inputs):
    x = np.asarray(inputs["x"], np.float32)
    cheb = np.asarray(inputs["cheb"], np.float32)
    theta1 = np.asarray(inputs["theta1"], np.float32)
    theta2 = np.asarray(inputs["theta2"], np.float32)
    mlp1_w = np.asarray(inputs["mlp1_w"], np.float32)
    mlp1_b = np.asarray(inputs["mlp1_b"], np.float32)
    mlp2_w = np.asarray(inputs["mlp2_w"], np.float32)
    mlp2_b = np.asarray(inputs["mlp2_b"], np.float32)
    tc1_w = np.asarray(inputs["tc1_w"], np.float32)
    tc1_b = np.asarray(inputs["tc1_b"], np.float32)
    tc2_w = np.asarray(inputs["tc2_w"], np.float32)
    tc2_b = np.asarray(inputs["tc2_b"], np.float32)
    res_w = np.asarray(inputs["res_w"], np.float32)
    res_b = np.asarray(inputs["res_b"], np.float32)
    ln_g = np.asarray(inputs["ln_g"], np.float32)
    ln_b = np.asarray(inputs["ln_b"], np.float32)

    assert np.array_equal(cheb[0], np.eye(N, dtype=np.float32)), \
        "kernel assumes cheb[0] == I"

    # SE attention on host: att[b,t] = sigmoid(relu(mean @ W1.T + b1) @ W2.T + b2)
    am = x.mean(axis=(2, 3))
    a1 = np.maximum(am @ mlp1_w.T + mlp1_b, 0.0)
    att = 1.0 / (1.0 + np.exp(-(a1 @ mlp2_w.T + mlp2_b)))
    att = att.astype(np.float32)

    c1b = cheb[1].astype(ml_dtypes.bfloat16)
    c2b = cheb[2].astype(ml_dtypes.bfloat16)

    th1 = np.zeros((3, 128, 256), np.float32)
    for g in range(8):
        for k in range(3):
            th1[k, g * 16:(g + 1) * 16, g * 32:(g + 1) * 32] = theta1[k]
    th2 = np.zeros((3, 128, 128), np.float32)
    for g in range(4):
        for k in range(3):
            th2[k, g * 32:(g + 1) * 32, g * 32:(g + 1) * 32] = theta2[k]
    th2b = th2[0].astype(ml_dtypes.bfloat16)

    src0 = [10] + list(range(11))
    src1 = [11] + list(range(1, 12))
    tcbd = np.zeros((2, 3, 3, 128, 128), np.float32)
    for ti, w in ((0, tc1_w), (1, tc2_w)):
        for tpp in range(12):
            jt, to = divmod(tpp, 4)
            for srcs, kw in ((src0, 0), (src1, 1)):
                tin = srcs[tpp]
                ji, til = divmod(tin, 4)
                tcbd[ti, jt, ji, til * 32:(til + 1) * 32,
                     to * 32:(to + 1) * 32] += w[:, :, 0, kw].T
    tcb1 = tcbd[0].astype(ml_dtypes.bfloat16)
    tcb2 = tcbd[1].astype(ml_dtypes.bfloat16)

    resbd = np.zeros((2, 128, 128), np.float32)
    for half in range(2):
        for g4 in range(4):
            g = g4 + 4 * half
            resbd[half, g * 16:(g + 1) * 16,
                  g4 * 32:(g4 + 1) * 32] = res_w[:, :, 0, 0].T

    identb = np.eye(128, dtype=ml_dtypes.bfloat16)
    lng = np.ascontiguousarray(np.broadcast_to(ln_g, (128, N))).astype(np.float32)
    lnb = np.ascontiguousarray(np.broadcast_to(ln_b, (128, N))).astype(np.float32)
    p32 = np.arange(128) % 32
    bias3 = np.stack([tc1_b[p32], tc2_b[p32], res_b[p32],
                      np.zeros(128, np.float32)], axis=1).astype(np.float32)

    shared = dict(c1b=c1b, c2b=c2b, th1=th1, th2=th2, th2b=th2b, tcb1=tcb1,
                  tcb2=tcb2, resbd=resbd, identb=identb, lng=lng, lnb=lnb,
                  bias3=bias3)

    in_maps = []
    for c in range(NCORES):
        xc = x[c * BC:(c + 1) * BC]                       # [4, 12, 16, N]
        xT = np.ascontiguousarray(
            xc.transpose(3, 0, 1, 2).reshape(N, R1)).astype(ml_dtypes.bfloat16)
        xb = np.ascontiguousarray(xc.reshape(R1, N))
        attc = np.zeros((128, 8), np.float32)
        for j in range(J1):
            for p in range(128):
                bt = 8 * j + p // 16
                attc[p, j] = att[c * BC + bt // T, bt % T]
        in_maps.append(dict(shared, xT=xT, xb=xb, attc=attc))
    return in_maps


def _zero_lnb(inputs):
    return bool(np.all(np.asarray(inputs["ln_b"]) == 0.0))


def kernel(**inputs):
    zl = _zero_lnb(inputs)
    if zl not in _compiled:
        _compiled[zl] = _build(zl)
    in_maps = _host_prep(inputs)
    res = run_bass_kernel_spmd(_compiled[zl], in_maps, list(range(NCORES)))
    y = np.empty((B, T, O, N), np.float32)
    for c in range(NCORES):
        y[c * BC:(c + 1) * BC] = res.results[c]["y"].reshape(BC, T, O, N)
    return y
